# revision 7
# baseline (speedup 1.0000x reference)
"""MoE block (B=16,N=1024,C=768,E=8,H=192,D=4,K=2) on 8 NeuronCores.

Strategy: data-parallel over B (2 samples/core). Per sample, noisy gating in
fp16 (fp32 PSUM), top-2 experts, one indirect-DMA gather of each chosen
expert's packed fp8 weights, then the 2-layer MLP entirely in fp8 DoubleRow
matmuls (2 contraction rows/partition, fp32 accumulate), exact Gelu on the
scalar engine, gate scaling fused into the h activations, channel-major fp16
output with the residual added from the fp16 x kept in SBUF. The [C, N]
output layout is untransposed on the host.

Host prep (pure value-preserving reshape/quantize): x shipped once as fp16
and once as fp8 in [128, 6, 1024] partition-major transposed layout; gate_w
gathered by task_id to fp16; fc1/fc2 weights packed per-expert into one fp8
row-block (x8 scale on fc1, x4 on fc2, undone on device) so one gather per
expert fetches everything incl. biases.
"""
import numpy as np
import ml_dtypes

import concourse.bass as bass
import concourse.mybir as mybir
import concourse.tile as tile
from concourse import bacc
from concourse.bass_utils import run_bass_kernel_spmd

bf16 = ml_dtypes.bfloat16
f16 = np.float16
f8 = ml_dtypes.float8_e4m3fn
f32 = np.float32
AF = mybir.ActivationFunctionType
ALU = mybir.AluOpType
DR = mybir.MatmulPerfMode.DoubleRow
dt = mybir.dt

B, N, C = 16, 1024, 768
E, H, D, TOPK = 8, 192, 4, 2
NCORES = 8
SPC = B // NCORES          # samples per core = 2
C_K = C // 128             # 6 chunks over channels
TCH = N // 128             # 8 token chunks
W1S, W2S = 8.0, 4.0        # fp8 weight scales (undone via act scale / gates)
# packed per-expert fp8 row layout (one indirect gather per expert):
# [0:1152)    fc1: k-chunk j at cols 192j..192j+192, row p = 8*W1[128j+p, h]
# [1152:1920) fc2 head: col 1152+c, row p = 4*W2[h=p, c]
# [1920:2688) fc2 tail: col 1920+c, row p<64 = 4*W2[h=128+p, c]; row 64 = 4*b2
# [2688:2690) fc1 bias: col 2688 row p = b1[p]; col 2689 row p<64 = b1[128+p]
PCK = 2690

_cache = {}


def _build(reps=1, general_bias=False):
    key = ("nc", reps, general_bias)
    if key in _cache:
        return _cache[key]
    nc = bacc.Bacc("TRN2", target_bir_lowering=False, debug=False,
                   num_devices=NCORES)

    x16_d = nc.dram_tensor("x16", [SPC, 128, C_K, N], dt.float16, kind="ExternalInput").ap()
    x8_d = nc.dram_tensor("x8", [SPC, 128, C_K, N], dt.float8e4, kind="ExternalInput").ap()
    gw_d = nc.dram_tensor("gw16", [SPC, 128, C_K, 2 * E], dt.float16, kind="ExternalInput").ap()
    ep_d = nc.dram_tensor("eps_r", [SPC, 128, TCH, E], dt.float32, kind="ExternalInput").ap()
    wp_d = nc.dram_tensor("wpack", [E * 128, PCK], dt.float8e4, kind="ExternalInput").ap()
    y_d = nc.dram_tensor("y", [SPC, 128, C_K, N], dt.float16, kind="ExternalOutput").ap()

    with tile.TileContext(nc) as tc:
        with tc.tile_pool(name="const", bufs=1) as cp, \
             tc.tile_pool(name="x16", bufs=2) as x16p, \
             tc.tile_pool(name="x8", bufs=2) as x8p, \
             tc.tile_pool(name="gin", bufs=2) as ginp, \
             tc.tile_pool(name="gate", bufs=2) as gp, \
             tc.tile_pool(name="wt", bufs=4) as wtp, \
             tc.tile_pool(name="h8", bufs=4) as h8p, \
             tc.tile_pool(name="g16", bufs=4) as g16p, \
             tc.tile_pool(name="ys", bufs=2) as ysp, \
             tc.tile_pool(name="ps_g", bufs=2, space="PSUM") as pgp, \
             tc.tile_pool(name="ps_t", bufs=2, space="PSUM") as ptp, \
             tc.tile_pool(name="ps_1", bufs=2, space="PSUM") as ps1p, \
             tc.tile_pool(name="ps_2", bufs=2, space="PSUM") as ps2p:

            # constants
            iota_i = cp.tile([128, 1], dt.int32, tag="iota_i")
            iota_f = cp.tile([128, 1], dt.float32, tag="iota_f")
            nc.gpsimd.iota(iota_i[:], pattern=[[0, 1]], base=0, channel_multiplier=1)
            nc.vector.tensor_copy(iota_f[:], iota_i[:])
            ones_r = cp.tile([1, 128], dt.float32, tag="ones_r")
            nc.vector.memset(ones_r[:], 1.0)
            ones_c = cp.tile([128, 1], dt.float32, tag="ones_c")
            nc.vector.memset(ones_c[:], 1.0)

            for rep in range(reps):
              # ---- A. issue all loads (sample 0 first so gating starts early)
              x16t, x8t, gwt, epst = [], [], [], []
              for s in range(SPC):
                  xt = x16p.tile([128, C_K, N], dt.float16, tag=f"x16_{s}")
                  nc.sync.dma_start(xt[:, :, :], x16_d[s, :, :, :])
                  gt = ginp.tile([128, C_K, 2 * E], dt.float16, tag=f"gw_{s}")
                  nc.sync.dma_start(gt[:, :, :], gw_d[s, :, :, :])
                  et = ginp.tile([128, TCH, E], dt.float32, tag=f"ep_{s}")
                  nc.sync.dma_start(et[:, :, :], ep_d[s, :, :, :])
                  x8 = x8p.tile([128, C_K, N], dt.float8e4, tag=f"x8_{s}")
                  nc.sync.dma_start(x8[:, :, :], x8_d[s, :, :, :])
                  x16t.append(xt); x8t.append(x8); gwt.append(gt); epst.append(et)

              # ---- B. gating per sample: fp16 matmuls tokens-major ----
              states = []
              for s in range(SPC):
                  gs = gp.tile([128, TCH, 2 * E], dt.float32, tag=f"gs{s}")
                  for t in range(TCH):
                      pg = pgp.tile([128, 2 * E], dt.float32, space="PSUM", tag="pg")
                      for k in range(C_K):
                          nc.tensor.matmul(
                              out=pg[:, :],
                              lhsT=x16t[s][:, k, 128 * t:128 * (t + 1)],
                              rhs=gwt[s][:, k, :],
                              start=(k == 0), stop=(k == C_K - 1))
                      nc.vector.tensor_copy(gs[:, t, :], pg[:, :])
                  # noise: eps * (softplus(raw) + 0.01), summed over tokens
                  ex = gp.tile([128, TCH, E], dt.float32, tag="ex")
                  nc.scalar.activation(ex[:, :, :], gs[:, :, E:2 * E], AF.Exp)
                  sp = gp.tile([128, TCH, E], dt.float32, tag="sp")
                  nc.scalar.activation(sp[:, :, :], ex[:, :, :], AF.Ln, bias=1.0)
                  nc.vector.tensor_scalar_add(sp[:, :, :], sp[:, :, :], 0.01)
                  prod = gp.tile([128, TCH, E], dt.float32, tag="prod")
                  nc.vector.tensor_tensor(out=prod[:, :, :], in0=sp[:, :, :],
                                          in1=epst[s][:, :, :], op=ALU.mult)
                  redp = gp.tile([128, E], dt.float32, tag="redp")
                  nc.vector.tensor_reduce(
                      out=redp[:, :], in_=prod[:, :, :].rearrange("p t e -> p e t"),
                      axis=mybir.AxisListType.X, op=ALU.add)
                  redc = gp.tile([128, E], dt.float32, tag="redc")
                  nc.vector.tensor_reduce(
                      out=redc[:, :], in_=gs[:, :, 0:E].rearrange("p t e -> p e t"),
                      axis=mybir.AxisListType.X, op=ALU.add)
                  ewsp = gp.tile([128, E], dt.float32, tag="ewsp")
                  nc.vector.tensor_add(ewsp[:, :], redp[:, :], redc[:, :])
                  # sum over 128 token partitions, then broadcast back to 128
                  ews_ps = ptp.tile([1, E], dt.float32, space="PSUM", tag="pt")
                  nc.tensor.matmul(out=ews_ps[:, :], lhsT=ones_c[:, :],
                                   rhs=ewsp[:, :], start=True, stop=True)
                  ews_row = gp.tile([1, E], dt.float32, tag="ews_row")
                  nc.vector.tensor_copy(ews_row[:], ews_ps[:])
                  bc_ps = ptp.tile([128, E], dt.float32, space="PSUM", tag="pt")
                  nc.tensor.matmul(out=bc_ps[:, :], lhsT=ones_r[:, :],
                                   rhs=ews_row[:, :], start=True, stop=True)
                  ewsb = gp.tile([128, E], dt.float32, tag="ewsb")
                  nc.vector.tensor_copy(ewsb[:], bc_ps[:])
                  # top-2 and gates (K=2 closed form, matches reference)
                  mx = gp.tile([128, E], dt.float32, tag="mx")
                  mi = gp.tile([128, E], dt.uint32, tag="mi")
                  nc.vector.max_with_indices(mx[:], mi[:], ewsb[:])
                  dd = gp.tile([128, 1], dt.float32, tag="dd")
                  nc.vector.tensor_sub(dd[:], mx[:, 0:1], mx[:, 1:2])
                  den = gp.tile([128, 1], dt.float32, tag="den")
                  nc.vector.tensor_scalar_add(den[:], dd[:], 1e-6)
                  rec = gp.tile([128, 1], dt.float32, tag="rec")
                  nc.vector.reciprocal(rec[:], den[:])
                  s1 = gp.tile([128, 1], dt.float32, tag="s1")
                  nc.vector.tensor_tensor(out=s1[:], in0=dd[:], in1=rec[:], op=ALU.mult)
                  # sigmoid via Exp to stay in the ln/exp activation table
                  et = gp.tile([128, 1], dt.float32, tag="et")
                  nc.scalar.activation(et[:], s1[:], AF.Exp, scale=-1.0)
                  den2 = gp.tile([128, 1], dt.float32, tag="den2")
                  nc.vector.tensor_scalar_add(den2[:], et[:], 1.0)
                  g1 = gp.tile([128, 1], dt.float32, tag="g1")
                  nc.vector.reciprocal(g1[:], den2[:])
                  g2 = gp.tile([128, 1], dt.float32, tag="g2")
                  nc.vector.tensor_tensor(out=g2[:], in0=et[:], in1=g1[:],
                                          op=ALU.mult)
                  gq = []
                  for j in range(TOPK):
                      gj = g1 if j == 0 else g2
                      gqj = gp.tile([128, 1], dt.float32, tag=f"gq{j}")
                      nc.vector.tensor_scalar_mul(gqj[:], gj[:], 1.0 / W2S)
                      gq.append(gqj)
                  # gather offsets: row = expert*128 + p
                  gis = []
                  for j in range(TOPK):
                      idxf = gp.tile([128, 1], dt.float32, tag=f"idxf{j}")
                      nc.vector.tensor_copy(idxf[:], mi[:, j:j + 1])
                      b1f = gp.tile([128, 1], dt.float32, tag=f"b1f{j}")
                      nc.vector.tensor_scalar_mul(b1f[:], idxf[:], 128.0)
                      nc.vector.tensor_add(b1f[:], b1f[:], iota_f[:])
                      gi = gp.tile([128, 1], dt.uint32, tag=f"gi{j}")
                      nc.vector.tensor_copy(gi[:], b1f[:])
                      gis.append(gi)
                  states.append((gq, gis))

              # ---- C. experts: gather fp8 weights, fc1 DoubleRow, gelu ----
              hstates = []
              for s in range(SPC):
                  gq, gis = states[s]
                  wts, h8s = [], []
                  for j in range(TOPK):
                      wt = wtp.tile([128, PCK], dt.float8e4, tag=f"wt{j}")
                      nc.gpsimd.indirect_dma_start(
                          out=wt[:], out_offset=None, in_=wp_d[:],
                          in_offset=bass.IndirectOffsetOnAxis(ap=gis[j][:, :1], axis=0))
                      w1v = wt[:, 0:6 * H].rearrange("p (k h) -> p k h", k=C_K)
                      h8 = h8p.tile([128, 2, N], dt.float8e4, tag=f"h8_{j}")
                      # zero the unused tail-pad rows of contraction group 1
                      nc.gpsimd.memset(h8[64:128, 1, :], 0.0)
                      if general_bias:
                          # fc2 bias rides the gathered 4*b2 row against g_j/4
                          nc.vector.tensor_copy(
                              h8[64:65, 1, :],
                              gq[j][0:1, 0:1].to_broadcast([1, N]))
                      for m in range(2):
                          msz = 128 if m == 0 else H - 128
                          for n in range(2):
                              ps1 = ps1p.tile([msz, 512], dt.float32, space="PSUM",
                                              tag="ps1")
                              for jp in range(C_K // 2):
                                  nc.tensor.matmul(
                                      out=ps1[:, :],
                                      lhsT=w1v[:, 2 * jp:2 * jp + 2,
                                               128 * m:128 * m + msz],
                                      rhs=x8t[s][:, 2 * jp:2 * jp + 2,
                                                 512 * n:512 * (n + 1)],
                                      start=(jp == 0), stop=(jp == C_K // 2 - 1),
                                      perf_mode=DR)
                              g16 = g16p.tile([msz, 512], dt.float16, tag="g16")
                              nc.scalar.activation(
                                  g16[:, :], ps1[:, :], AF.Gelu,
                                  bias=wt[0:msz, 2688 + m:2689 + m],
                                  scale=1.0 / W1S)
                              tgt = (h8[:, 0, 512 * n:512 * (n + 1)] if m == 0
                                     else h8[0:msz, 1, 512 * n:512 * (n + 1)])
                              eng = nc.vector if m == 0 else nc.gpsimd
                              eng.tensor_scalar_mul(tgt, g16[:, :],
                                                    gq[j][0:msz, :])
                      wts.append(wt); h8s.append(h8)
                  hstates.append((wts, h8s))

              # ---- D. fc2 DoubleRow + residual + store ----
              for s in range(SPC):
                  wts, h8s = hstates[s]
                  ys = ysp.tile([128, C_K, N], dt.float16, tag="ys")
                  w2v = [wt[:, 6 * H:6 * H + 2 * C].rearrange("p (g c) -> p g c", g=2)
                         for wt in wts]
                  for cc in range(C_K):
                      for n in range(2):
                          ps2 = ps2p.tile([128, 512], dt.float32, space="PSUM",
                                          tag="ps2")
                          for j in range(TOPK):
                              nc.tensor.matmul(
                                  out=ps2[:, :],
                                  lhsT=w2v[j][:, :, 128 * cc:128 * (cc + 1)],
                                  rhs=h8s[j][:, :, 512 * n:512 * (n + 1)],
                                  start=(j == 0), stop=(j == TOPK - 1),
                                  perf_mode=DR)
                          nc.vector.tensor_tensor(
                              out=ys[:, cc, 512 * n:512 * (n + 1)],
                              in0=ps2[:, :],
                              in1=x16t[s][:, cc, 512 * n:512 * (n + 1)],
                              op=ALU.add)
                      if cc % 2 == 1:
                          nc.sync.dma_start(y_d[s, :, cc - 1:cc + 1, :],
                                            ys[:, cc - 1:cc + 1, :])

    nc.compile()
    _cache[key] = nc
    return nc


def _prep_inputs(x, task_ids, eps, gate_w, fc1_w, fc1_b, fc2_w, fc2_b):
    x = np.asarray(x, dtype=f32)
    task_ids = np.asarray(task_ids).astype(np.int64)
    eps = np.asarray(eps, dtype=f32)
    gate_w = np.asarray(gate_w, dtype=f32)
    fc1_w = np.asarray(fc1_w, dtype=f32)
    fc1_b = np.asarray(fc1_b, dtype=f32)
    fc2_w = np.asarray(fc2_w, dtype=f32)
    fc2_b = np.asarray(fc2_b, dtype=f32)

    # x transposed to [B, 128, 6, 1024]: partition p holds channels 128j+p
    xT = np.ascontiguousarray(
        x.transpose(0, 2, 1).reshape(B, C_K, 128, N).transpose(0, 2, 1, 3))
    x16 = xT.astype(f16)
    x8 = xT.astype(f8)

    gw = gate_w[task_ids]                                  # [B, C, 2E]
    gw16 = np.ascontiguousarray(
        gw.reshape(B, C_K, 128, 2 * E).transpose(0, 2, 1, 3)).astype(f16)

    eps_r = np.ascontiguousarray(
        eps.reshape(B, TCH, 128, E).transpose(0, 2, 1, 3))  # [B,128,8,8]

    w1T = fc1_w.transpose(0, 2, 1)                         # [E, C, H]
    w2T = fc2_w.transpose(0, 2, 1)                         # [E, H, C]
    wpack = np.zeros((E, 128, PCK), dtype=f32)
    for j in range(C_K):
        wpack[:, :, H * j:H * (j + 1)] = W1S * w1T[:, 128 * j:128 * (j + 1), :]
    wpack[:, :, 1152:1920] = W2S * w2T[:, 0:128, :]
    wpack[:, 0:64, 1920:2688] = W2S * w2T[:, 128:H, :]
    wpack[:, 64, 1920:2688] = W2S * fc2_b
    wpack[:, :, 2688] = fc1_b[:, 0:128]
    wpack[:, 0:64, 2689] = fc1_b[:, 128:H]
    wpack = wpack.reshape(E * 128, PCK).astype(f8)

    general_bias = bool(np.any(fc2_b))

    in_maps = []
    for c in range(NCORES):
        sl = slice(SPC * c, SPC * (c + 1))
        in_maps.append({
            "x16": x16[sl], "x8": x8[sl], "gw16": gw16[sl],
            "eps_r": eps_r[sl], "wpack": wpack,
        })
    return in_maps, general_bias


def kernel(x, task_ids, eps, gate_w, fc1_w, fc1_b, fc2_w, fc2_b, _trace=False):
    in_maps, general_bias = _prep_inputs(
        x, task_ids, eps, gate_w, fc1_w, fc1_b, fc2_w, fc2_b)
    nc = _build(general_bias=general_bias)
    res = run_bass_kernel_spmd(nc, in_maps, list(range(NCORES)), trace=_trace)
    y = np.concatenate([res.results[c]["y"] for c in range(NCORES)], axis=0)
    kernel.last_results = res
    # [B, 128, 6, 1024] -> [B, N, C] with c = 128j + p
    out = y.astype(np.float32).transpose(0, 3, 2, 1).reshape(B, N, C)
    return np.ascontiguousarray(out)
